# revision 3
# baseline (speedup 1.0000x reference)
"""CNN-LSTM Trainium2 kernel (nn_CNNLSTM_59193239273595), v2.

Data-parallel over 8 NeuronCores: batch 64 -> 8 sequences per core.

Key structural changes vs v1 (2.03ms):
  * No xg precompute / evacuation: the LSTM input projection is folded
    into the per-step PSUM accumulation as 4 extra matmuls (K=65: the
    65th row is a ones-row carrying the bias).  This removes ~220us of
    DVE evacuation work and ~60us of tensor work that collided with the
    early LSTM steps.
  * Embedding gather in 512-row slices with no chunk overlap; conv tap
    matmuls read across slice boundaries (small tail matmuls).  First
    slice lands in ~27us so the LSTM starts early; the gather's 8-queue
    DMA stream paces the conv work naturally under the running LSTM.
  * Conv relu+bias runs on the vector engine (tensor_scalar add+max),
    keeping the scalar queue pure sigmoid/tanh (it is on the LSTM's
    critical cycle).
  * PSUM: conv pool 2 banks + LSTM pool 6 banks, so the per-step psum
    tile recycle never stalls the chain.

Per step and group (4 seqs), the critical cycle is
  h -> 4 whh matmuls -> sigmoid(16 cols, one ACT via tanh(g)=2sig(2g)-1
  with the 2x folded into host-side weights) -> m,fcv,C (DVE) ->
  tanh (ACT) -> h (DVE), two groups staggered.
"""

import sys
from contextlib import ExitStack

if "/opt/trn_rl_repo" not in sys.path:
    sys.path.insert(0, "/opt/trn_rl_repo")

import numpy as np
import ml_dtypes

import concourse.bass as bass
import concourse.tile as tile
from concourse import bacc, mybir
from concourse.bass_utils import run_bass_kernel_spmd

BF16 = ml_dtypes.bfloat16

# Problem shapes (hardcoded per contract).
B, L = 64, 4096
VOCAB, E, F, K, P, H, C = 20000, 128, 64, 5, 4, 128, 2
NCORES = 8
S = B // NCORES          # sequences per core
LC = L - K + 1           # 4092
T = LC // P              # 1023
NB = 8                   # gather slices / conv blocks per sequence
SLICE = 512              # embedding columns per gather slice
KW = F + 1               # contraction rows for the xg matmuls (ones-row)

F32 = mybir.dt.float32
BF = mybir.dt.bfloat16
I16 = mybir.dt.int16

AF = mybir.ActivationFunctionType
OP = mybir.AluOpType


def _block_cols(b):
    """Conv output columns of block b (pre-pool)."""
    return 512 if b < NB - 1 else LC - 512 * (NB - 1)   # 508 for b=7


def _block_T(b):
    return _block_cols(b) // P                           # 128 / 127


def build_nc():
    nc = bacc.Bacc("TRN2", target_bir_lowering=False, debug=False)

    # ---- DRAM I/O ----
    x_idx_d = nc.dram_tensor("x_idx", [S * NB, 128, SLICE // 16], I16,
                             kind="ExternalInput")
    emb_d = nc.dram_tensor("emb_bf", [VOCAB, E], BF, kind="ExternalInput")
    convT_d = nc.dram_tensor("convT", [K, E, F], BF, kind="ExternalInput")
    convb_d = nc.dram_tensor("convb", [F, 1], F32, kind="ExternalInput")
    wihbT_d = nc.dram_tensor("wihbT", [4, KW, H], BF, kind="ExternalInput")
    whhT_d = nc.dram_tensor("whhT", [4, H, H], BF, kind="ExternalInput")
    fcwT_d = nc.dram_tensor("fcwT", [H, C], BF, kind="ExternalInput")
    fcb_d = nc.dram_tensor("fcb", [C, 1], F32, kind="ExternalInput")
    out_d = nc.dram_tensor("out", [C, S], F32, kind="ExternalOutput")

    with tile.TileContext(nc) as tc, ExitStack() as st:
        wp = st.enter_context(tc.tile_pool(name="weights", bufs=1))
        idxp = st.enter_context(tc.tile_pool(name="idx", bufs=8))
        embp = st.enter_context(tc.tile_pool(name="emb", bufs=18))
        cop = st.enter_context(tc.tile_pool(name="convout", bufs=1))
        mpp = st.enter_context(tc.tile_pool(name="mp", bufs=4))
        stp = st.enter_context(tc.tile_pool(name="state", bufs=1))
        outp = st.enter_context(tc.tile_pool(name="outp", bufs=1))

        # ---- load weights to SBUF ----
        convT_sb = wp.tile([E, K * F], BF, tag="convT")
        for k in range(K):
            nc.sync.dma_start(convT_sb[:, k * F:(k + 1) * F], convT_d.ap()[k])
        convb_sb = wp.tile([F, 1], F32, tag="convb")
        nc.sync.dma_start(convb_sb[:], convb_d.ap()[:])
        wihbT_sb = wp.tile([KW, 4 * H], BF, tag="wihbT")
        for g in range(4):
            nc.sync.dma_start(wihbT_sb[:, g * H:(g + 1) * H], wihbT_d.ap()[g])
        whhT_sb = wp.tile([H, 4 * H], BF, tag="whhT")
        for g in range(4):
            nc.sync.dma_start(whhT_sb[:, g * H:(g + 1) * H], whhT_d.ap()[g])
        fcwT_sb = wp.tile([H, C], BF, tag="fcwT")
        nc.sync.dma_start(fcwT_sb[:], fcwT_d.ap()[:])
        fcb_sb = wp.tile([C, 1], F32, tag="fcb")
        nc.sync.dma_start(fcb_sb[:], fcb_d.ap()[:])

        # ---- conv output tiles: [KW, T_b, S] bf16, row 64 = ones (bias) ----
        co = []
        for b in range(NB):
            t = cop.tile([KW, _block_T(b) * S], BF, tag=f"co{b}", name=f"co{b}")
            co.append(t[:].rearrange("p (t s) -> p t s", s=S))
            nc.vector.memset(t[F:KW, :], 1.0)

        with (
            tc.tile_pool(name="cvps", bufs=2, space="PSUM") as cvps,
            tc.tile_pool(name="lstmps", bufs=6, space="PSUM") as lps,
            tc.tile_pool(name="sigs", bufs=6) as sgp,
            tc.tile_pool(name="ltmp", bufs=8) as ltp,
        ):
            # ---- gathers + conv blocks ----
            embs = {}

            def emit_gather(s, b):
                idx_t = idxp.tile([128, SLICE // 16], I16, tag="idx")
                nc.sync.dma_start(idx_t[:], x_idx_d.ap()[s * NB + b])
                embT = embp.tile([128, 1, SLICE], BF, tag="embT")
                nc.gpsimd.dma_gather(
                    embT[:], emb_d.ap()[:], idx_t[:], SLICE, SLICE, E,
                    transpose=True, single_packet=False,
                )
                embs[(s, b)] = embT

            def emit_conv(s, b):
                w = _block_cols(b)
                ps = cvps.tile([F, 512], F32, tag="cvps", name="cv_ps")
                if b < NB - 1:
                    # main taps from slice b, boundary tails from slice b+1
                    for k in range(K):
                        nc.tensor.matmul(
                            ps[:, 0:512 - k],
                            convT_sb[:, k * F:(k + 1) * F],
                            embs[(s, b)][:, 0, k:512],
                            start=(k == 0), stop=False,
                        )
                    for k in range(1, K):
                        nc.tensor.matmul(
                            ps[:, 512 - k:512],
                            convT_sb[:, k * F:(k + 1) * F],
                            embs[(s, b + 1)][:, 0, 0:k],
                            start=False, stop=(k == K - 1),
                        )
                else:
                    for k in range(K):
                        nc.tensor.matmul(
                            ps[:, 0:w],
                            convT_sb[:, k * F:(k + 1) * F],
                            embs[(s, b)][:, 0, k:k + w],
                            start=(k == 0), stop=(k == K - 1),
                        )
                # maxpool(4) via tensor_reduce, two halves
                tb = _block_T(b)
                mp = mpp.tile([F, 128], F32, tag="mp", name="mp_t")
                half = (tb + 1) // 2
                for hh in range(2):
                    c0 = hh * half
                    c1 = min(tb, (hh + 1) * half)
                    if c1 <= c0:
                        continue
                    nc.vector.tensor_reduce(
                        mp[:, c0:c1],
                        ps[:, c0 * P:c1 * P].rearrange("p (a b) -> p a b", b=P),
                        axis=mybir.AxisListType.X,
                        op=OP.max,
                    )
                # relu + bias on DVE; strided write into co[b][:, :, s]
                nc.vector.tensor_scalar(
                    co[b][0:F, :, s],
                    mp[:, 0:tb],
                    convb_sb[:, 0:1],
                    0.0,
                    OP.add,
                    op1=OP.max,
                )

            for b in range(NB):
                for s in range(S):
                    emit_gather(s, b)
                    if b >= 1:
                        emit_conv(s, b - 1)
            for s in range(S):
                emit_conv(s, NB - 1)

            # ---- LSTM ----
            c_states = [
                stp.tile([H, 4], F32, tag="c_state_a", name="c_state_a"),
                stp.tile([H, 4], F32, tag="c_state_b", name="c_state_b"),
            ]
            h_states = [
                stp.tile([H, 4], BF, tag="h_state_a", name="h_state_a"),
                stp.tile([H, 4], BF, tag="h_state_b", name="h_state_b"),
            ]
            for grp in range(2):
                nc.vector.memset(c_states[grp][:], 0.0)
                nc.vector.memset(h_states[grp][:], 0.0)

            def head(grp, t):
                b, tt = divmod(t, 128)
                ps = lps.tile([128, 16], F32, tag="lstmps")
                rhs_x = co[b][:, tt, grp * 4:(grp + 1) * 4]
                for g in range(4):
                    nc.tensor.matmul(
                        ps[:, g * 4:(g + 1) * 4],
                        wihbT_sb[:, g * H:(g + 1) * H],
                        rhs_x,
                        start=(g == 0), stop=False,
                    )
                for g in range(4):
                    nc.tensor.matmul(
                        ps[:, g * 4:(g + 1) * 4],
                        whhT_sb[:, g * H:(g + 1) * H],
                        h_states[grp][:],
                        start=False, stop=(g == 3),
                    )
                sg = sgp.tile([128, 16], F32, tag="sigs")
                nc.scalar.activation(sg[:], ps[:], AF.Sigmoid)
                m = ltp.tile([H, 4], F32, tag="m")
                nc.vector.scalar_tensor_tensor(
                    m[:], sg[:, 12:16], 0.5, sg[:, 0:4], OP.subtract, OP.mult,
                )
                fcv = ltp.tile([H, 4], F32, tag="fcv")
                nc.vector.tensor_mul(fcv[:], sg[:, 4:8], c_states[grp][:])
                nc.vector.scalar_tensor_tensor(
                    c_states[grp][:], m[:], 2.0, fcv[:], OP.mult, OP.add,
                )
                return sg

            def tail(grp, sg):
                tch_t = ltp.tile([H, 4], F32, tag="tc")
                nc.scalar.activation(tch_t[:], c_states[grp][:], AF.Tanh)
                nc.vector.tensor_mul(h_states[grp][:], sg[:, 8:12], tch_t[:])

            pending = {}
            for t in range(T):
                for grp in range(2):
                    sg = head(grp, t)
                    other = 1 - grp
                    if other in pending:
                        tail(other, pending.pop(other))
                    pending[grp] = sg
            for grp, sg in sorted(pending.items()):
                tail(grp, sg)

            # ---- FC ----
            psf = lps.tile([C, 16], F32, tag="lstmps")
            for grp in range(2):
                nc.tensor.matmul(
                    psf[:, grp * 4:(grp + 1) * 4],
                    fcwT_sb[:],
                    h_states[grp][:],
                    start=(grp == 0),
                    stop=(grp == 1),
                )
            out_sb = outp.tile([C, S], F32, tag="out")
            nc.scalar.activation(
                out_sb[:], psf[:, :8], AF.Identity, bias=fcb_sb[:, 0:1]
            )
            nc.sync.dma_start(out_d.ap()[:], out_sb[:])

    nc.compile()
    return nc


def prep_inputs(x, emb, conv_w, conv_b, w_ih, w_hh, b_ih, b_hh, fc_w, fc_b):
    """Host-side prep: per-core in_maps for run_bass_kernel_spmd."""
    x = np.asarray(x)
    emb = np.asarray(emb, np.float32)
    conv_w = np.asarray(conv_w, np.float32)
    conv_b = np.asarray(conv_b, np.float32)
    w_ih = np.asarray(w_ih, np.float32)
    w_hh = np.asarray(w_hh, np.float32)
    b_ih = np.asarray(b_ih, np.float32)
    b_hh = np.asarray(b_hh, np.float32)
    fc_w = np.asarray(fc_w, np.float32)
    fc_b = np.asarray(fc_b, np.float32)

    # gate order [i, f, o, g]; the "g" gate row-block is scaled by 2 for the
    # tanh(x) = 2*sigmoid(2x) - 1 trick.
    slices = [slice(0, H), slice(H, 2 * H), slice(3 * H, 4 * H), slice(2 * H, 3 * H)]
    scales = [1.0, 1.0, 1.0, 2.0]

    whhT = np.stack(
        [(w_hh[sl] * sc).T.astype(BF16) for sl, sc in zip(slices, scales)]
    )  # [4, H, H]
    # augmented input projection: [4, KW=65, H]; row 64 = bias row
    wihbT = np.zeros((4, KW, H), BF16)
    for g, (sl, sc) in enumerate(zip(slices, scales)):
        wihbT[g, :F, :] = (w_ih[sl] * sc).T.astype(BF16)
        wihbT[g, F, :] = ((b_ih + b_hh)[sl] * sc).astype(BF16)

    convT = np.stack(
        [conv_w[:, :, k].T.astype(BF16) for k in range(K)]
    )  # [K, E, F]

    shared = {
        "emb_bf": emb.astype(BF16),
        "convT": convT,
        "convb": conv_b.astype(np.float32)[:, None],
        "wihbT": wihbT,
        "whhT": whhT,
        "fcwT": fc_w.T.astype(BF16),
        "fcb": fc_b.astype(np.float32)[:, None],
    }

    # gather positions: slice b covers l in [b*512, b*512+512)
    pos = (np.arange(NB)[:, None] * SLICE + np.arange(SLICE)[None, :])  # [NB, 512]

    in_maps = []
    for c in range(NCORES):
        xc = np.asarray(x[c * S:(c + 1) * S], np.int64)       # [S, L]
        toks = xc[:, pos]                                     # [S, NB, 512]
        # wrapped layout: idx i lives at [i % 16, i // 16], replicated over
        # the 8 groups of 16 partitions.
        xr = toks.reshape(S, NB, SLICE // 16, 16).transpose(0, 1, 3, 2)
        x_idx = np.tile(xr, (1, 1, 8, 1)).astype(np.int16)    # [S,NB,128,32]
        in_maps.append(
            {"x_idx": x_idx.reshape(S * NB, 128, SLICE // 16), **shared}
        )
    return in_maps


_NC_CACHE = {}


def _get_nc():
    if "nc" not in _NC_CACHE:
        _NC_CACHE["nc"] = build_nc()
    return _NC_CACHE["nc"]


def _assemble(results):
    out = np.zeros((B, C), np.float32)
    for c in range(NCORES):
        out[c * S:(c + 1) * S] = results[c]["out"].T
    return out


def run(inputs, trace=False):
    nc = _get_nc()
    in_maps = prep_inputs(**inputs)
    res = run_bass_kernel_spmd(nc, in_maps, list(range(NCORES)), trace=trace)
    return _assemble(res.results), res


def kernel(**inputs) -> np.ndarray:
    out, _ = run(inputs)
    return out


# revision 5
# speedup vs baseline: 1.0305x; 1.0305x over previous
"""CNN-LSTM Trainium2 kernel (nn_CNNLSTM_59193239273595), v3.1.

Data-parallel over 8 NeuronCores: batch 64 -> 8 sequences per core.

Structure:
  * Embedding gather in 512-row slices, offset-0 tiles (dma_gather does
    NOT honor offset destination APs).  Conv block b reads only slice b
    with 5 uniform 508-wide tap matmuls -> pooled steps [128b, 128b+127).
    The 7 slice-boundary pooled steps (t = 128j-1) come from one extra
    128-row gather per sequence that packs the 7 eight-token boundary
    windows; each boundary is 5 tiny 4-wide tap matmuls.
  * Conv results land in per-(block, group) tiles [65, 4 seqs, T] bf16
    (contiguous relu+bias writes on DVE), row 64 = ones so the bias
    rides the K=65 input-projection matmuls.
  * No xg precompute: the input projection is 4 extra K=65 matmuls
    accumulated into the per-step PSUM tile ahead of the 4 recurrent
    matmuls; only the recurrent ones sit on the critical cycle.
  * The recurrent path per step+group: h -> 4 whh matmuls -> sigmoid
    (one ACT for all 4 gates via tanh(g)=2sig(2g)-1 folded into host
    weights) -> m,fcv,C on DVE -> tanh -> h; two groups staggered.
  * Conv work drips into the LSTM emission (2 closures/step) so the
    scheduler spreads it into the cycle's dead air.
"""

import sys
from contextlib import ExitStack

if "/opt/trn_rl_repo" not in sys.path:
    sys.path.insert(0, "/opt/trn_rl_repo")

import numpy as np
import ml_dtypes

import concourse.bass as bass
import concourse.tile as tile
from concourse import bacc, mybir
from concourse.bass_utils import run_bass_kernel_spmd

BF16 = ml_dtypes.bfloat16

# Problem shapes (hardcoded per contract).
B, L = 64, 4096
VOCAB, E, F, K, P, H, C = 20000, 128, 64, 5, 4, 128, 2
NCORES = 8
S = B // NCORES          # sequences per core
LC = L - K + 1           # 4092
T = LC // P              # 1023
NB = 8                   # gather slices / conv blocks per sequence
SLICE = 512
MW = 508                 # main conv block width (pre-pool)
MT = 127                 # pooled steps per main block
KW = F + 1               # contraction rows for the xg matmuls (ones-row)

F32 = mybir.dt.float32
BF = mybir.dt.bfloat16
I16 = mybir.dt.int16

AF = mybir.ActivationFunctionType
OP = mybir.AluOpType


def build_nc():
    nc = bacc.Bacc("TRN2", target_bir_lowering=False, debug=False)

    # ---- DRAM I/O ----
    x_idx_d = nc.dram_tensor("x_idx", [S * NB, 128, SLICE // 16], I16,
                             kind="ExternalInput")
    bidx_d = nc.dram_tensor("bidx", [S, 128, 8], I16, kind="ExternalInput")
    emb_d = nc.dram_tensor("emb_bf", [VOCAB, E], BF, kind="ExternalInput")
    convT_d = nc.dram_tensor("convT", [K, E, F], BF, kind="ExternalInput")
    convb_d = nc.dram_tensor("convb", [F, 1], F32, kind="ExternalInput")
    wihbT_d = nc.dram_tensor("wihbT", [4, KW, H], BF, kind="ExternalInput")
    whhT_d = nc.dram_tensor("whhT", [4, H, H], BF, kind="ExternalInput")
    fcwT_d = nc.dram_tensor("fcwT", [H, C], BF, kind="ExternalInput")
    fcb_d = nc.dram_tensor("fcb", [C, 1], F32, kind="ExternalInput")
    out_d = nc.dram_tensor("out", [C, S], F32, kind="ExternalOutput")

    with tile.TileContext(nc) as tc, ExitStack() as st:
        wp = st.enter_context(tc.tile_pool(name="weights", bufs=1))
        idxp = st.enter_context(tc.tile_pool(name="idx", bufs=8))
        embp = st.enter_context(tc.tile_pool(name="emb", bufs=18))
        bep = st.enter_context(tc.tile_pool(name="bemb", bufs=1))
        cop = st.enter_context(tc.tile_pool(name="convout", bufs=1))
        mpp = st.enter_context(tc.tile_pool(name="mp", bufs=4))
        stp = st.enter_context(tc.tile_pool(name="state", bufs=1))
        outp = st.enter_context(tc.tile_pool(name="outp", bufs=1))

        # ---- load weights to SBUF ----
        convT_sb = wp.tile([E, K * F], BF, tag="convT")
        for k in range(K):
            nc.sync.dma_start(convT_sb[:, k * F:(k + 1) * F], convT_d.ap()[k])
        convb_sb = wp.tile([F, 1], F32, tag="convb")
        nc.sync.dma_start(convb_sb[:], convb_d.ap()[:])
        wihbT_sb = wp.tile([KW, 4 * H], BF, tag="wihbT")
        for g in range(4):
            nc.sync.dma_start(wihbT_sb[:, g * H:(g + 1) * H], wihbT_d.ap()[g])
        whhT_sb = wp.tile([H, 4 * H], BF, tag="whhT")
        for g in range(4):
            nc.sync.dma_start(whhT_sb[:, g * H:(g + 1) * H], whhT_d.ap()[g])
        fcwT_sb = wp.tile([H, C], BF, tag="fcwT")
        nc.sync.dma_start(fcwT_sb[:], fcwT_d.ap()[:])
        fcb_sb = wp.tile([C, 1], F32, tag="fcb")
        nc.sync.dma_start(fcb_sb[:], fcb_d.ap()[:])

        # ---- conv output tiles ----
        # main: per (block, group) [KW, 4, 127]; boundary: per (j, group)
        # [KW, 4, 1] holding pooled step 128j-1.  Row 64 = ones.
        com = [[None, None] for _ in range(NB)]
        cob = [[None, None] for _ in range(NB)]    # index j-1 -> step 128j-1
        for b in range(NB):
            for grp in range(2):
                t = cop.tile([KW, 4 * MT], BF, tag=f"com{b}g{grp}",
                             name=f"com{b}g{grp}")
                com[b][grp] = t[:].rearrange("p (s t) -> p s t", s=4)
                nc.vector.memset(t[F:KW, :], 1.0)
        for j in range(1, NB):
            for grp in range(2):
                t = cop.tile([KW, 4], BF, tag=f"cob{j}g{grp}",
                             name=f"cob{j}g{grp}")
                cob[j - 1][grp] = t[:].rearrange("p (s t) -> p s t", s=4)
                nc.vector.memset(t[F:KW, :], 1.0)

        with (
            tc.tile_pool(name="cvps", bufs=2, space="PSUM") as cvps,
            tc.tile_pool(name="lstmps", bufs=6, space="PSUM") as lps,
            tc.tile_pool(name="sigs", bufs=6) as sgp,
            tc.tile_pool(name="ltmp", bufs=8) as ltp,
        ):
            embs = {}
            bexts = {}

            def emit_gather(s, b):
                idx_t = idxp.tile([128, SLICE // 16], I16, tag="idx")
                nc.sync.dma_start(idx_t[:], x_idx_d.ap()[s * NB + b])
                embT = embp.tile([128, 1, SLICE], BF, tag="embT")
                nc.gpsimd.dma_gather(
                    embT[:], emb_d.ap()[:], idx_t[:], SLICE, SLICE, E,
                    transpose=True, single_packet=False,
                )
                embs[(s, b)] = embT

            def emit_bgather(s):
                idx_t = idxp.tile([128, 8], I16, tag="idx")
                nc.sync.dma_start(idx_t[:], bidx_d.ap()[s])
                bt = bep.tile([128, 1, 128], BF, tag=f"bext{s}")
                nc.gpsimd.dma_gather(
                    bt[:], emb_d.ap()[:], idx_t[:], 128, 128, E,
                    transpose=True, single_packet=False,
                )
                bexts[s] = bt

            def conv_closures(s, b):
                """Main conv block: 5 taps x 508 cols -> 127 pooled steps."""
                grp, sl = divmod(s, 4)
                state = {}
                cl = []

                def mk_mm(k):
                    def f():
                        if k == 0:
                            state["ps"] = cvps.tile([F, 512], F32, tag="cvps",
                                                    name="cv_ps")
                        nc.tensor.matmul(
                            state["ps"][:, 0:MW],
                            convT_sb[:, k * F:(k + 1) * F],
                            embs[(s, b)][:, 0, k:k + MW],
                            start=(k == 0), stop=(k == K - 1),
                        )
                    return f

                def mk_red(hh):
                    def f():
                        mp = state.setdefault(
                            "mp", mpp.tile([F, 128], F32, tag="mp", name="mp_t"))
                        c0, c1 = (0, 64) if hh == 0 else (64, MT)
                        nc.vector.tensor_reduce(
                            mp[:, c0:c1],
                            state["ps"][:, c0 * P:c1 * P]
                                .rearrange("p (a b) -> p a b", b=P),
                            axis=mybir.AxisListType.X,
                            op=OP.max,
                        )
                    return f

                def mk_relu():
                    def f():
                        nc.vector.tensor_scalar(
                            com[b][grp][0:F, sl, 0:MT],
                            state["mp"][:, 0:MT],
                            convb_sb[:, 0:1],
                            0.0,
                            OP.add,
                            op1=OP.max,
                        )
                    return f

                for k in range(K):
                    cl.append(mk_mm(k))
                cl.append(mk_red(0))
                cl.append(mk_red(1))
                cl.append(mk_relu())
                return cl

            def bnd_closures(s, j):
                """Boundary block j: pooled step 128j-1 (conv cols
                [512j-4, 512j)) from the packed boundary gather."""
                grp, sl = divmod(s, 4)
                state = {}
                cl = []
                base = (j - 1) * 8

                def mk_mm(k):
                    def f():
                        if k == 0:
                            state["ps"] = cvps.tile([F, 512], F32, tag="cvps",
                                                    name="cvb_ps")
                        nc.tensor.matmul(
                            state["ps"][:, 0:4],
                            convT_sb[:, k * F:(k + 1) * F],
                            bexts[s][:, 0, base + k:base + k + 4],
                            start=(k == 0), stop=(k == K - 1),
                        )
                    return f

                def mk_tail():
                    def f():
                        mp = mpp.tile([F, 128], F32, tag="mp", name="mpb_t")
                        nc.vector.tensor_reduce(
                            mp[:, 0:1],
                            state["ps"][:, 0:4]
                                .rearrange("p (a b) -> p a b", b=P),
                            axis=mybir.AxisListType.X,
                            op=OP.max,
                        )
                        nc.vector.tensor_scalar(
                            cob[j - 1][grp][0:F, sl, 0:1],
                            mp[:, 0:1],
                            convb_sb[:, 0:1],
                            0.0,
                            OP.add,
                            op1=OP.max,
                        )
                    return f

                for k in range(K):
                    cl.append(mk_mm(k))
                cl.append(mk_tail())
                return cl

            for s in range(S):
                emit_gather(s, 0)
            for s in range(S):
                emit_gather(s, 1)
            for s in range(S):
                emit_bgather(s)
            for b in range(2, NB):
                for s in range(S):
                    emit_gather(s, b)

            # block 0 computed up front (lead-in)
            for s in range(S):
                for f in conv_closures(s, 0):
                    f()

            # drip schedule: main block (s,b) from step ~128(b-1)+4+10s;
            # boundary (s,j) right after its round.
            sched = {}
            for b in range(1, NB):
                for s in range(S):
                    sched.setdefault(128 * (b - 1) + 4 + 10 * s, []).append(
                        ("m", s, b))
            for j in range(1, NB):
                for s in range(S):
                    sched.setdefault(128 * (j - 1) + 88 + 4 * s, []).append(
                        ("b", s, j))

            # ---- LSTM ----
            c_states = [
                stp.tile([H, 4], F32, tag="c_state_a", name="c_state_a"),
                stp.tile([H, 4], F32, tag="c_state_b", name="c_state_b"),
            ]
            h_states = [
                stp.tile([H, 4], BF, tag="h_state_a", name="h_state_a"),
                stp.tile([H, 4], BF, tag="h_state_b", name="h_state_b"),
            ]
            for grp in range(2):
                nc.vector.memset(c_states[grp][:], 0.0)
                nc.vector.memset(h_states[grp][:], 0.0)

            def head(grp, t):
                b, lt = divmod(t, 128)
                if lt == 127:
                    rhs_x = cob[b][grp][:, :, 0]
                else:
                    rhs_x = com[b][grp][:, :, lt]
                ps = lps.tile([128, 16], F32, tag="lstmps")
                for g in range(4):
                    nc.tensor.matmul(
                        ps[:, g * 4:(g + 1) * 4],
                        wihbT_sb[:, g * H:(g + 1) * H],
                        rhs_x,
                        start=(g == 0), stop=False,
                    )
                for g in range(4):
                    nc.tensor.matmul(
                        ps[:, g * 4:(g + 1) * 4],
                        whhT_sb[:, g * H:(g + 1) * H],
                        h_states[grp][:],
                        start=False, stop=(g == 3),
                    )
                sg = sgp.tile([128, 16], F32, tag="sigs")
                nc.scalar.activation(sg[:], ps[:], AF.Sigmoid)
                m = ltp.tile([H, 4], F32, tag="m")
                nc.vector.scalar_tensor_tensor(
                    m[:], sg[:, 12:16], 0.5, sg[:, 0:4], OP.subtract, OP.mult,
                )
                fcv = ltp.tile([H, 4], F32, tag="fcv")
                nc.vector.tensor_mul(fcv[:], sg[:, 4:8], c_states[grp][:])
                nc.vector.scalar_tensor_tensor(
                    c_states[grp][:], m[:], 2.0, fcv[:], OP.mult, OP.add,
                )
                return sg

            def tail(grp, sg):
                tch_t = ltp.tile([H, 4], F32, tag="tc")
                nc.scalar.activation(tch_t[:], c_states[grp][:], AF.Tanh)
                nc.vector.tensor_mul(h_states[grp][:], sg[:, 8:12], tch_t[:])

            live = []
            pending = {}
            for t in range(T):
                for key in sched.get(t, []):
                    kind, s, b = key
                    live.append(conv_closures(s, b) if kind == "m"
                                else bnd_closures(s, b))
                for grp in range(2):
                    sg = head(grp, t)
                    other = 1 - grp
                    if other in pending:
                        tail(other, pending.pop(other))
                    pending[grp] = sg
                budget = 2
                while budget > 0 and live:
                    live[0].pop(0)()
                    if not live[0]:
                        live.pop(0)
                    budget -= 1
            while live:
                live[0].pop(0)()
                if not live[0]:
                    live.pop(0)
            for grp, sg in sorted(pending.items()):
                tail(grp, sg)

            # ---- FC ----
            psf = lps.tile([C, 16], F32, tag="lstmps")
            for grp in range(2):
                nc.tensor.matmul(
                    psf[:, grp * 4:(grp + 1) * 4],
                    fcwT_sb[:],
                    h_states[grp][:],
                    start=(grp == 0),
                    stop=(grp == 1),
                )
            out_sb = outp.tile([C, S], F32, tag="out")
            nc.scalar.activation(
                out_sb[:], psf[:, :8], AF.Identity, bias=fcb_sb[:, 0:1]
            )
            nc.sync.dma_start(out_d.ap()[:], out_sb[:])

    nc.compile()
    return nc


def prep_inputs(x, emb, conv_w, conv_b, w_ih, w_hh, b_ih, b_hh, fc_w, fc_b):
    """Host-side prep: per-core in_maps for run_bass_kernel_spmd."""
    x = np.asarray(x)
    emb = np.asarray(emb, np.float32)
    conv_w = np.asarray(conv_w, np.float32)
    conv_b = np.asarray(conv_b, np.float32)
    w_ih = np.asarray(w_ih, np.float32)
    w_hh = np.asarray(w_hh, np.float32)
    b_ih = np.asarray(b_ih, np.float32)
    b_hh = np.asarray(b_hh, np.float32)
    fc_w = np.asarray(fc_w, np.float32)
    fc_b = np.asarray(fc_b, np.float32)

    # gate order [i, f, o, g]; the "g" gate row-block is scaled by 2 for the
    # tanh(x) = 2*sigmoid(2x) - 1 trick.
    slices = [slice(0, H), slice(H, 2 * H), slice(3 * H, 4 * H), slice(2 * H, 3 * H)]
    scales = [1.0, 1.0, 1.0, 2.0]

    whhT = np.stack(
        [(w_hh[sl] * sc).T.astype(BF16) for sl, sc in zip(slices, scales)]
    )  # [4, H, H]
    # augmented input projection: [4, KW=65, H]; row 64 = bias row
    wihbT = np.zeros((4, KW, H), BF16)
    for g, (sl, sc) in enumerate(zip(slices, scales)):
        wihbT[g, :F, :] = (w_ih[sl] * sc).T.astype(BF16)
        wihbT[g, F, :] = ((b_ih + b_hh)[sl] * sc).astype(BF16)

    convT = np.stack(
        [conv_w[:, :, k].T.astype(BF16) for k in range(K)]
    )  # [K, E, F]

    shared = {
        "emb_bf": emb.astype(BF16),
        "convT": convT,
        "convb": conv_b.astype(np.float32)[:, None],
        "wihbT": wihbT,
        "whhT": whhT,
        "fcwT": fc_w.T.astype(BF16),
        "fcb": fc_b.astype(np.float32)[:, None],
    }

    def wrap(tokens):
        """[..., n] -> [..., 128, n//16] wrapped-idx layout."""
        n = tokens.shape[-1]
        tr = tokens.reshape(*tokens.shape[:-1], n // 16, 16)
        tr = np.swapaxes(tr, -1, -2)                      # [..., 16, n//16]
        reps = (1,) * (tr.ndim - 2) + (8, 1)
        return np.tile(tr, reps).astype(np.int16)         # [..., 128, n//16]

    # main slices: slice b covers l in [b*512, b*512+512)
    pos = (np.arange(NB)[:, None] * SLICE + np.arange(SLICE)[None, :])
    # boundary windows: j=1..7 -> l in [512j-4, 512j+4), packed 8 per j,
    # padded to 128 with token 0
    bpos = np.concatenate(
        [np.arange(512 * j - 4, 512 * j + 4) for j in range(1, NB)]
        + [np.zeros(128 - 56, np.int64)]
    )  # [128]

    in_maps = []
    for c in range(NCORES):
        xc = np.asarray(x[c * S:(c + 1) * S], np.int64)       # [S, L]
        x_idx = wrap(xc[:, pos].reshape(S, NB, SLICE))        # [S,NB,128,32]
        bidx = wrap(xc[:, bpos])                              # [S,128,8]
        in_maps.append({
            "x_idx": x_idx.reshape(S * NB, 128, SLICE // 16),
            "bidx": bidx,
            **shared,
        })
    return in_maps


_NC_CACHE = {}


def _get_nc():
    if "nc" not in _NC_CACHE:
        _NC_CACHE["nc"] = build_nc()
    return _NC_CACHE["nc"]


def _assemble(results):
    out = np.zeros((B, C), np.float32)
    for c in range(NCORES):
        out[c * S:(c + 1) * S] = results[c]["out"].T
    return out


def run(inputs, trace=False):
    nc = _get_nc()
    in_maps = prep_inputs(**inputs)
    res = run_bass_kernel_spmd(nc, in_maps, list(range(NCORES)), trace=trace)
    return _assemble(res.results), res


def kernel(**inputs) -> np.ndarray:
    out, _ = run(inputs)
    return out


# revision 6
# speedup vs baseline: 1.1180x; 1.0849x over previous
"""CNN-LSTM Trainium2 kernel (nn_CNNLSTM_59193239273595), v4.

Data-parallel over 8 NeuronCores: batch 64 -> 8 sequences per core.

Structure:
  * Embedding gather in 512-row slices, offset-0 tiles.  Conv block b
    reads only slice b (5 uniform 508-wide tap matmuls -> pooled steps
    [128b, 128b+127)).  The 7 slice-boundary pooled steps (t = 128j-1)
    come from one extra 128-row gather per sequence packing the 7
    eight-token boundary windows (5 tiny tap matmuls each).
  * Conv results land in per-(block, group) tiles [65, 4 seqs, T] bf16;
    row 64 = ones so the bias rides the K=65 input-projection matmuls.
  * No xg precompute: the input projection is 4 extra K=65 matmuls
    accumulated into the per-step PSUM tile ahead of the 4 recurrent
    matmuls.
  * All-tanh cell: every gate preact is evaluated as tau = tanh(pre/2)
    = 2*sigmoid(pre)-1 (tanh(g) directly for the g gate), with the
    /2 and the doubled cell/hidden state (D = 2c, h2 = 2h) folded into
    host-side weight scales (all powers of two, exact in bf16).  Per
    step+group the elementwise tail is then:
      pair = (tau_{i,f} + 1) * [tau_g, D_prev]   (ONE dual-width STT)
      D    = 0.5*pair_f + pair_g                 (STT, -> next ring tile)
      tc   = tanh(0.5 * D)                       (ACT, scale slot)
      h2   = (tau_o + 1) * tc                    (STT)
    which drops one DVE op + one cross-wait from the recurrence cycle
    vs the classic m/fcv/C form.  D lives in cols 16:20 of the next
    step's sg ring tile so the pair op reads [tau_g | D] as one AP.
  * Conv work is paced into the LSTM's dead air with REAL dependencies:
    each conv closure (one engine op) gets a semaphore dep on the
    sigma/tanh ACT of the step where it is dripped (1 closure/step,
    windows sized so every tile is ready ~15 steps before consumption).
"""

import sys
from contextlib import ExitStack

if "/opt/trn_rl_repo" not in sys.path:
    sys.path.insert(0, "/opt/trn_rl_repo")

import numpy as np
import ml_dtypes

import concourse.bass as bass
import concourse.tile as tile
from concourse import bacc, mybir
from concourse.bass_utils import run_bass_kernel_spmd

BF16 = ml_dtypes.bfloat16

# Problem shapes (hardcoded per contract).
B, L = 64, 4096
VOCAB, E, F, K, P, H, C = 20000, 128, 64, 5, 4, 128, 2
NCORES = 8
S = B // NCORES          # sequences per core
LC = L - K + 1           # 4092
T = LC // P              # 1023
NB = 8                   # gather slices / conv blocks per sequence
SLICE = 512
MW = 508                 # main conv block width (pre-pool)
MT = 127                 # pooled steps per main block
KW = F + 1               # contraction rows for the xg matmuls (ones-row)

F32 = mybir.dt.float32
BF = mybir.dt.bfloat16
I16 = mybir.dt.int16

AF = mybir.ActivationFunctionType
OP = mybir.AluOpType


def build_nc():
    nc = bacc.Bacc("TRN2", target_bir_lowering=False, debug=False)

    # ---- DRAM I/O ----
    x_idx_d = nc.dram_tensor("x_idx", [S * NB, 128, SLICE // 16], I16,
                             kind="ExternalInput")
    bidx_d = nc.dram_tensor("bidx", [S, 128, 8], I16, kind="ExternalInput")
    emb_d = nc.dram_tensor("emb_bf", [VOCAB, E], BF, kind="ExternalInput")
    convT_d = nc.dram_tensor("convT", [K, E, F], BF, kind="ExternalInput")
    convb_d = nc.dram_tensor("convb", [F, 1], F32, kind="ExternalInput")
    wihbT_d = nc.dram_tensor("wihbT", [4, KW, H], BF, kind="ExternalInput")
    whhT_d = nc.dram_tensor("whhT", [4, H, H], BF, kind="ExternalInput")
    fcwT_d = nc.dram_tensor("fcwT", [H, C], BF, kind="ExternalInput")
    fcb_d = nc.dram_tensor("fcb", [C, 1], F32, kind="ExternalInput")
    out_d = nc.dram_tensor("out", [C, S], F32, kind="ExternalOutput")

    with tile.TileContext(nc) as tc, ExitStack() as st:
        wp = st.enter_context(tc.tile_pool(name="weights", bufs=1))
        idxp = st.enter_context(tc.tile_pool(name="idx", bufs=8))
        embp = st.enter_context(tc.tile_pool(name="emb", bufs=18))
        bep = st.enter_context(tc.tile_pool(name="bemb", bufs=1))
        cop = st.enter_context(tc.tile_pool(name="convout", bufs=1))
        mpp = st.enter_context(tc.tile_pool(name="mp", bufs=4))
        stp = st.enter_context(tc.tile_pool(name="state", bufs=1))
        outp = st.enter_context(tc.tile_pool(name="outp", bufs=1))

        # ---- load weights to SBUF ----
        convT_sb = wp.tile([E, K * F], BF, tag="convT")
        for k in range(K):
            nc.sync.dma_start(convT_sb[:, k * F:(k + 1) * F], convT_d.ap()[k])
        convb_sb = wp.tile([F, 1], F32, tag="convb")
        nc.sync.dma_start(convb_sb[:], convb_d.ap()[:])
        wihbT_sb = wp.tile([KW, 4 * H], BF, tag="wihbT")
        for g in range(4):
            nc.sync.dma_start(wihbT_sb[:, g * H:(g + 1) * H], wihbT_d.ap()[g])
        whhT_sb = wp.tile([H, 4 * H], BF, tag="whhT")
        for g in range(4):
            nc.sync.dma_start(whhT_sb[:, g * H:(g + 1) * H], whhT_d.ap()[g])
        fcwT_sb = wp.tile([H, C], BF, tag="fcwT")
        nc.sync.dma_start(fcwT_sb[:], fcwT_d.ap()[:])
        fcb_sb = wp.tile([C, 1], F32, tag="fcb")
        nc.sync.dma_start(fcb_sb[:], fcb_d.ap()[:])

        # ---- conv output tiles ----
        com = [[None, None] for _ in range(NB)]
        cob = [[None, None] for _ in range(NB)]    # index j-1 -> step 128j-1
        for b in range(NB):
            for grp in range(2):
                t = cop.tile([KW, 4 * MT], BF, tag=f"com{b}g{grp}",
                             name=f"com{b}g{grp}")
                com[b][grp] = t[:].rearrange("p (s t) -> p s t", s=4)
                nc.vector.memset(t[F:KW, :], 1.0)
        for j in range(1, NB):
            for grp in range(2):
                t = cop.tile([KW, 4], BF, tag=f"cob{j}g{grp}",
                             name=f"cob{j}g{grp}")
                cob[j - 1][grp] = t[:].rearrange("p (s t) -> p s t", s=4)
                nc.vector.memset(t[F:KW, :], 1.0)

        # ---- sg ring: [128, 20] per (group, slot): cols 0:16 = tau gates
        # [i,f,o,g], cols 16:20 = D (written by the PREVIOUS step) ----
        NRING = 4
        sgr = [[wp.tile([128, 20], F32, tag=f"sgr{g}_{i}", name=f"sgr{g}_{i}")
                for i in range(NRING)] for g in range(2)]

        with (
            tc.tile_pool(name="cvps", bufs=2, space="PSUM") as cvps,
            tc.tile_pool(name="lstmps", bufs=6, space="PSUM") as lps,
            tc.tile_pool(name="ltmp", bufs=8) as ltp,
        ):
            embs = {}
            bexts = {}
            pace = {"inst": None}

            def _paced(inst):
                if pace["inst"] is not None:
                    bass._add_dep_helper(
                        inst.ins, pace["inst"].ins, sync=True,
                        reason="conv pacing")
                return inst

            def emit_gather(s, b):
                idx_t = idxp.tile([128, SLICE // 16], I16, tag="idx")
                nc.sync.dma_start(idx_t[:], x_idx_d.ap()[s * NB + b])
                embT = embp.tile([128, 1, SLICE], BF, tag="embT")
                nc.gpsimd.dma_gather(
                    embT[:], emb_d.ap()[:], idx_t[:], SLICE, SLICE, E,
                    transpose=True, single_packet=False,
                )
                embs[(s, b)] = embT

            def emit_bgather(s):
                idx_t = idxp.tile([128, 8], I16, tag="idx")
                nc.sync.dma_start(idx_t[:], bidx_d.ap()[s])
                bt = bep.tile([128, 1, 128], BF, tag=f"bext{s}")
                nc.gpsimd.dma_gather(
                    bt[:], emb_d.ap()[:], idx_t[:], 128, 128, E,
                    transpose=True, single_packet=False,
                )
                bexts[s] = bt

            def conv_closures(s, b, paced=True):
                """Main conv block: 5 taps x 508 cols -> 127 pooled steps."""
                grp, sl = divmod(s, 4)
                state = {}
                cl = []
                dep = _paced if paced else (lambda i: i)

                def mk_mm(k):
                    def f():
                        if k == 0:
                            state["ps"] = cvps.tile([F, 512], F32, tag="cvps",
                                                    name="cv_ps")
                        dep(nc.tensor.matmul(
                            state["ps"][:, 0:MW],
                            convT_sb[:, k * F:(k + 1) * F],
                            embs[(s, b)][:, 0, k:k + MW],
                            start=(k == 0), stop=(k == K - 1),
                        ))
                    return f

                def mk_red(hh):
                    def f():
                        mp = state.setdefault(
                            "mp", mpp.tile([F, 128], F32, tag="mp", name="mp_t"))
                        c0, c1 = (0, 64) if hh == 0 else (64, MT)
                        dep(nc.vector.tensor_reduce(
                            mp[:, c0:c1],
                            state["ps"][:, c0 * P:c1 * P]
                                .rearrange("p (a b) -> p a b", b=P),
                            axis=mybir.AxisListType.X,
                            op=OP.max,
                        ))
                    return f

                def mk_relu():
                    def f():
                        dep(nc.vector.tensor_scalar(
                            com[b][grp][0:F, sl, 0:MT],
                            state["mp"][:, 0:MT],
                            convb_sb[:, 0:1],
                            0.0,
                            OP.add,
                            op1=OP.max,
                        ))
                    return f

                for k in range(K):
                    cl.append(mk_mm(k))
                cl.append(mk_red(0))
                cl.append(mk_red(1))
                cl.append(mk_relu())
                return cl

            def bnd_closures(s, j):
                """Boundary block j: pooled step 128j-1."""
                grp, sl = divmod(s, 4)
                state = {}
                cl = []
                base = (j - 1) * 8

                def mk_mm(k):
                    def f():
                        if k == 0:
                            state["ps"] = cvps.tile([F, 512], F32, tag="cvps",
                                                    name="cvb_ps")
                        _paced(nc.tensor.matmul(
                            state["ps"][:, 0:4],
                            convT_sb[:, k * F:(k + 1) * F],
                            bexts[s][:, 0, base + k:base + k + 4],
                            start=(k == 0), stop=(k == K - 1),
                        ))
                    return f

                def mk_tail():
                    def f():
                        mp = mpp.tile([F, 128], F32, tag="mp", name="mpb_t")
                        _paced(nc.vector.tensor_reduce(
                            mp[:, 0:1],
                            state["ps"][:, 0:4]
                                .rearrange("p (a b) -> p a b", b=P),
                            axis=mybir.AxisListType.X,
                            op=OP.max,
                        ))
                        _paced(nc.vector.tensor_scalar(
                            cob[j - 1][grp][0:F, sl, 0:1],
                            mp[:, 0:1],
                            convb_sb[:, 0:1],
                            0.0,
                            OP.add,
                            op1=OP.max,
                        ))
                    return f

                for k in range(K):
                    cl.append(mk_mm(k))
                cl.append(mk_tail())
                return cl

            for s in range(S):
                emit_gather(s, 0)
            for s in range(S):
                emit_gather(s, 1)
            for s in range(S):
                emit_bgather(s)
            for b in range(2, NB):
                for s in range(S):
                    emit_gather(s, b)

            # block 0 computed up front (lead-in, unpaced)
            for s in range(S):
                for f in conv_closures(s, 0, paced=False):
                    f()

            # drip schedule: window w covers steps [128(w-1), 128w) and
            # carries main block (s, b=w) + boundary (s, j=w); 1 closure
            # per step, all done >= 12 steps before consumption.
            sched = {}
            for w in range(1, NB):
                base_t = 128 * (w - 1)
                for s in range(S):
                    sched.setdefault(base_t + 2 + 14 * s, []).append(("m", s, w))
                    sched.setdefault(base_t + 10 + 14 * s, []).append(("b", s, w))

            # ---- LSTM ----
            c_init = True
            h_states = [
                stp.tile([H, 4], BF, tag="h_state_a", name="h_state_a"),
                stp.tile([H, 4], BF, tag="h_state_b", name="h_state_b"),
            ]
            for grp in range(2):
                nc.vector.memset(h_states[grp][:], 0.0)
                nc.vector.memset(sgr[grp][0][:, 16:20], 0.0)

            def head(grp, t):
                b, lt = divmod(t, 128)
                if lt == 127:
                    rhs_x = cob[b][grp][:, :, 0]
                else:
                    rhs_x = com[b][grp][:, :, lt]
                cur = sgr[grp][t % NRING]
                nxt = sgr[grp][(t + 1) % NRING]
                ps = lps.tile([128, 16], F32, tag="lstmps")
                for g in range(4):
                    nc.tensor.matmul(
                        ps[:, g * 4:(g + 1) * 4],
                        wihbT_sb[:, g * H:(g + 1) * H],
                        rhs_x,
                        start=(g == 0), stop=False,
                    )
                for g in range(4):
                    nc.tensor.matmul(
                        ps[:, g * 4:(g + 1) * 4],
                        whhT_sb[:, g * H:(g + 1) * H],
                        h_states[grp][:],
                        start=False, stop=(g == 3),
                    )
                act = nc.scalar.activation(cur[:, 0:16], ps[:], AF.Tanh)
                pace["inst"] = act
                pairt = ltp.tile([H, 8], F32, tag="pair")
                # [m2 | fD2] = (tau_{i,f} + 1) * [tau_g | D_prev]
                nc.vector.scalar_tensor_tensor(
                    pairt[:].rearrange("p (a b) -> p a b", b=4),
                    cur[:, 0:8].rearrange("p (a b) -> p a b", b=4),
                    1.0,
                    cur[:, 12:20].rearrange("p (a b) -> p a b", b=4),
                    OP.add, OP.mult,
                )
                # D_new = 0.5*fD2 + m2  -> next ring tile cols 16:20
                nc.vector.scalar_tensor_tensor(
                    nxt[:, 16:20], pairt[:, 4:8], 0.5, pairt[:, 0:4],
                    OP.mult, OP.add,
                )
                return cur, nxt

            def tail(grp, cur, nxt):
                tch_t = ltp.tile([H, 4], F32, tag="tc")
                nc.scalar.activation(tch_t[:], nxt[:, 16:20], AF.Tanh,
                                     scale=0.5)
                # h2 = (tau_o + 1) * tanh(c)
                nc.vector.scalar_tensor_tensor(
                    h_states[grp][:], cur[:, 8:12], 1.0, tch_t[:],
                    OP.add, OP.mult,
                )

            live = []
            pending = {}
            for t in range(T):
                for key in sched.get(t, []):
                    kind, s, b = key
                    live.append(conv_closures(s, b) if kind == "m"
                                else bnd_closures(s, b))
                for grp in range(2):
                    cur_nxt = head(grp, t)
                    other = 1 - grp
                    if other in pending:
                        tail(other, *pending.pop(other))
                    pending[grp] = cur_nxt
                if live:
                    live[0].pop(0)()
                    if not live[0]:
                        live.pop(0)
            while live:
                live[0].pop(0)()
                if not live[0]:
                    live.pop(0)
            for grp, cur_nxt in sorted(pending.items()):
                tail(grp, *cur_nxt)

            # ---- FC (h2 = 2h; fcwT pre-scaled by 0.5) ----
            psf = lps.tile([C, 16], F32, tag="lstmps")
            for grp in range(2):
                nc.tensor.matmul(
                    psf[:, grp * 4:(grp + 1) * 4],
                    fcwT_sb[:],
                    h_states[grp][:],
                    start=(grp == 0),
                    stop=(grp == 1),
                )
            out_sb = outp.tile([C, S], F32, tag="out")
            nc.scalar.activation(
                out_sb[:], psf[:, :8], AF.Identity, bias=fcb_sb[:, 0:1]
            )
            nc.sync.dma_start(out_d.ap()[:], out_sb[:])

    nc.compile()
    return nc


def prep_inputs(x, emb, conv_w, conv_b, w_ih, w_hh, b_ih, b_hh, fc_w, fc_b):
    """Host-side prep: per-core in_maps for run_bass_kernel_spmd."""
    x = np.asarray(x)
    emb = np.asarray(emb, np.float32)
    conv_w = np.asarray(conv_w, np.float32)
    conv_b = np.asarray(conv_b, np.float32)
    w_ih = np.asarray(w_ih, np.float32)
    w_hh = np.asarray(w_hh, np.float32)
    b_ih = np.asarray(b_ih, np.float32)
    b_hh = np.asarray(b_hh, np.float32)
    fc_w = np.asarray(fc_w, np.float32)
    fc_b = np.asarray(fc_b, np.float32)

    # gate order [i, f, o, g].  All gates evaluated as tau = tanh(pre/2)
    # (= 2*sigmoid(pre)-1) except g which is tanh(pre).  The /2 for
    # i,f,o and the doubled hidden state h2=2h (so whh gets another /2)
    # are folded here; all scales are powers of two (exact in bf16).
    slices = [slice(0, H), slice(H, 2 * H), slice(3 * H, 4 * H), slice(2 * H, 3 * H)]
    in_scales = [0.5, 0.5, 0.5, 1.0]

    whhT = np.stack(
        [(w_hh[sl] * (sc * 0.5)).T.astype(BF16)
         for sl, sc in zip(slices, in_scales)]
    )  # [4, H, H]
    wihbT = np.zeros((4, KW, H), BF16)
    for g, (sl, sc) in enumerate(zip(slices, in_scales)):
        wihbT[g, :F, :] = (w_ih[sl] * sc).T.astype(BF16)
        wihbT[g, F, :] = ((b_ih + b_hh)[sl] * sc).astype(BF16)

    convT = np.stack(
        [conv_w[:, :, k].T.astype(BF16) for k in range(K)]
    )  # [K, E, F]

    shared = {
        "emb_bf": emb.astype(BF16),
        "convT": convT,
        "convb": conv_b.astype(np.float32)[:, None],
        "wihbT": wihbT,
        "whhT": whhT,
        "fcwT": (fc_w.T * 0.5).astype(BF16),
        "fcb": fc_b.astype(np.float32)[:, None],
    }

    def wrap(tokens):
        """[..., n] -> [..., 128, n//16] wrapped-idx layout."""
        n = tokens.shape[-1]
        tr = tokens.reshape(*tokens.shape[:-1], n // 16, 16)
        tr = np.swapaxes(tr, -1, -2)                      # [..., 16, n//16]
        reps = (1,) * (tr.ndim - 2) + (8, 1)
        return np.tile(tr, reps).astype(np.int16)         # [..., 128, n//16]

    pos = (np.arange(NB)[:, None] * SLICE + np.arange(SLICE)[None, :])
    bpos = np.concatenate(
        [np.arange(512 * j - 4, 512 * j + 4) for j in range(1, NB)]
        + [np.zeros(128 - 56, np.int64)]
    )  # [128]

    in_maps = []
    for c in range(NCORES):
        xc = np.asarray(x[c * S:(c + 1) * S], np.int64)       # [S, L]
        x_idx = wrap(xc[:, pos].reshape(S, NB, SLICE))        # [S,NB,128,32]
        bidx = wrap(xc[:, bpos])                              # [S,128,8]
        in_maps.append({
            "x_idx": x_idx.reshape(S * NB, 128, SLICE // 16),
            "bidx": bidx,
            **shared,
        })
    return in_maps


_NC_CACHE = {}


def _get_nc():
    if "nc" not in _NC_CACHE:
        _NC_CACHE["nc"] = build_nc()
    return _NC_CACHE["nc"]


def _assemble(results):
    out = np.zeros((B, C), np.float32)
    for c in range(NCORES):
        out[c * S:(c + 1) * S] = results[c]["out"].T
    return out


def run(inputs, trace=False):
    nc = _get_nc()
    in_maps = prep_inputs(**inputs)
    res = run_bass_kernel_spmd(nc, in_maps, list(range(NCORES)), trace=trace)
    return _assemble(res.results), res


def kernel(**inputs) -> np.ndarray:
    out, _ = run(inputs)
    return out
